# revision 1
# baseline (speedup 1.0000x reference)
"""MLA (multi-head latent attention) Trainium2 kernel, 8-way sharded.

Strategy (tensor-parallel over heads per the DeepSeek-TP hint, plus
token-parallel stage 1):
  - stage 1 (token-parallel): each core owns 512 tokens (256 from each
    batch). Computes q_a = rmsnorm(x @ Wqa), the rmsnormed compressed
    latent c_kv, and the rotated shared rope key — all in transposed
    (feature x token) layout so no on-device transposes are needed.
  - AllGather the bf16 latents across the 8 cores.
  - stage 2 (head-TP, 2 heads/core): Wqb / Wkvb projections, RoPE on q,
    causal attention with max-free softmax (scores are bounded ~3, so
    exp never overflows), denominator via on-chip partition-sum matmul.
  - AllToAll attention outputs (each core receives all heads x its own
    tokens), then a token-sharded full-contraction Wo matmul.
All heavy matmuls run in bf16 (fp32 is 4 cycles/row on the PE; bf16 is
1 cycle/row). Statistics matmuls (partition sums / broadcasts) run as
float32r (1 cycle/row at free dim >= 256, ~FP22 precision).
"""

import numpy as np
import ml_dtypes

import concourse.bass as bass
import concourse.mybir as mybir
import concourse.tile as tile
from concourse import bacc
from concourse.bass_utils import run_bass_kernel_spmd

BF16 = mybir.dt.bfloat16
F32 = mybir.dt.float32
F32R = mybir.dt.float32r
AF = mybir.ActivationFunctionType

NCORES = 8
B, S, D = 2, 2048, 2048
H = 16
DN, DR, DV = 128, 64, 128
KVR, QR = 512, 1536
T = B * S            # 4096 flattened tokens
TS = T // NCORES     # 512 tokens per core (256 per batch)
HB = TS // B         # 256 tokens per batch per core
HL = H // NCORES     # 2 heads per core
EPS = 1.1920929e-7
BASE = 10000.0
SCALE = 1.0 / float(np.sqrt(DN + DR))

NT = 512             # token free-dim tile
P = 128
KD = D // P          # 16 contraction tiles over model dim
MQ = QR // P         # 12
MC = KVR // P        # 4
NTT = T // NT        # 8 global token n-tiles
CKV = KVR + DR       # 576
KO = H * DV // P     # 16 contraction tiles for Wo
SB = S // NT         # 4 qt tiles per batch

_CACHE = {}


# ---------------------------------------------------------------- host side

def _deint_perm():
    return np.concatenate([np.arange(0, DR, 2), np.arange(1, DR, 2)])


def _rope_tables():
    t = np.arange(S, dtype=np.float32)
    inv = 1.0 / (BASE ** (np.arange(0, DR, 2, dtype=np.float32) / DR))
    ang = np.outer(t, inv)
    emb = np.concatenate([ang, ang], axis=-1)          # (S, DR)
    return np.cos(emb), np.sin(emb)


def _shard_tokens(c):
    b0 = np.arange(HB * c, HB * (c + 1))
    return np.concatenate([b0, S + b0])


def host_prep(inputs):
    x = np.asarray(inputs["x"], np.float32).reshape(T, D)
    Wqa = np.asarray(inputs["Wqa"], np.float32)
    gqa = np.asarray(inputs["gqa"], np.float32)
    Wqb = np.asarray(inputs["Wqb"], np.float32)
    Wkva = np.asarray(inputs["Wkva"], np.float32)
    gkva = np.asarray(inputs["gkva"], np.float32)
    Wkvb = np.asarray(inputs["Wkvb"], np.float32)
    Wo = np.asarray(inputs["Wo"], np.float32)

    bf = lambda a: np.ascontiguousarray(a).astype(ml_dtypes.bfloat16)
    perm = _deint_perm()

    wqa_b = bf(Wqa)
    wkva_p = Wkva.copy()
    wkva_p[:, KVR:] = Wkva[:, KVR:][:, perm]
    wkva_b = bf(wkva_p)

    cos, sin = _rope_tables()                          # (S, 64)
    cosT, sinT = cos.T, sin.T                          # (64, S)
    cos_g = np.concatenate([cosT, cosT], axis=1)       # (64, T) cols = b*S+s
    sin_g = np.concatenate([sinT, sinT], axis=1)
    cos2 = bf(np.concatenate([cos_g, cos_g], axis=0))  # (128, T)
    sin2 = bf(np.concatenate([sin_g, sin_g], axis=0))

    # diagonal-tile masks, flattened to (128, 4*NT): block jm at cols jm*NT
    qcol = np.arange(NT)
    rr = np.arange(P)
    masks = np.zeros((P, 4 * NT), np.float32)
    for jm in range(4):
        masks[:, jm * NT:(jm + 1) * NT] = (
            qcol[None, :] >= (128 * jm + rr)[:, None])
    masks_b = bf(masks)

    ones_col = np.ones((P, 1), np.float32)
    ones_row = np.ones((1, P), np.float32)
    # rotate-half as a matmul: out = rotm.T @ in  (block-diag over 2 heads)
    # out[dout] = -in[dout+HD] for dout<HD, +in[dout-HD] for dout>=HD
    HD = DR // 2
    rotm = np.zeros((P, P), np.float32)
    for bb in (0, DR):
        for dout in range(HD):
            rotm[bb + dout + HD, bb + dout] = -1.0
        for dout in range(HD, DR):
            rotm[bb + dout - HD, bb + dout] = 1.0
    rotm_b = bf(rotm)
    wo_b = bf(Wo)

    in_maps = []
    for c in range(NCORES):
        toks = _shard_tokens(c)
        xT = bf(x[toks].T)                             # (2048, 512)
        h0, h1 = HL * c, HL * c + 1

        blk0 = Wqb[:, h0 * (DN + DR):(h0 + 1) * (DN + DR)]
        blk1 = Wqb[:, h1 * (DN + DR):(h1 + 1) * (DN + DR)]
        wqb_c = np.concatenate(
            [blk0[:, :DN], blk1[:, :DN],
             blk0[:, DN:][:, perm], blk1[:, DN:][:, perm]], axis=1)
        wqb_c = bf(wqb_c * gqa[:, None] * SCALE)       # (1536, 384)

        kb0 = Wkvb[:, h0 * (DN + DV):(h0 + 1) * (DN + DV)]
        kb1 = Wkvb[:, h1 * (DN + DV):(h1 + 1) * (DN + DV)]
        wkvbk_c = bf(np.concatenate([kb0[:, :DN], kb1[:, :DN]], axis=1)
                     * gkva[:, None])
        wkvbv_c = bf(np.concatenate([kb0[:, DN:], kb1[:, DN:]], axis=1)
                     * gkva[:, None])

        pos = toks % S
        cos_s1 = np.ascontiguousarray(cosT[:, pos]).astype(np.float32)
        sin_s1 = np.ascontiguousarray(sinT[:, pos]).astype(np.float32)

        in_maps.append({
            "xT": xT, "wqa": wqa_b, "wkva": wkva_b, "wqb": wqb_c,
            "wkvbk": wkvbk_c, "wkvbv": wkvbv_c, "wo": wo_b,
            "cos2": cos2, "sin2": sin2, "cos_s1": cos_s1, "sin_s1": sin_s1,
            "masks": masks_b, "ones_col": ones_col, "ones_row": ones_row,
            "rotm": rotm_b,
        })
    return in_maps


# ---------------------------------------------------------------- device IR

def build_nc(do_compile=True):
    nc = bacc.Bacc(
        "TRN2", target_bir_lowering=False, debug=False,
        enable_asserts=True, num_devices=NCORES,
    )
    xT = nc.dram_tensor("xT", [D, TS], BF16, kind="ExternalInput")
    wqa = nc.dram_tensor("wqa", [D, QR], BF16, kind="ExternalInput")
    wkva = nc.dram_tensor("wkva", [D, CKV], BF16, kind="ExternalInput")
    wqb = nc.dram_tensor("wqb", [QR, HL * (DN + DR)], BF16, kind="ExternalInput")
    wkvbk = nc.dram_tensor("wkvbk", [KVR, HL * DN], BF16, kind="ExternalInput")
    wkvbv = nc.dram_tensor("wkvbv", [KVR, HL * DV], BF16, kind="ExternalInput")
    wo = nc.dram_tensor("wo", [H * DV, D], BF16, kind="ExternalInput")
    cos2 = nc.dram_tensor("cos2", [P, T], BF16, kind="ExternalInput")
    sin2 = nc.dram_tensor("sin2", [P, T], BF16, kind="ExternalInput")
    cos_s1 = nc.dram_tensor("cos_s1", [DR, TS], F32, kind="ExternalInput")
    sin_s1 = nc.dram_tensor("sin_s1", [DR, TS], F32, kind="ExternalInput")
    masks = nc.dram_tensor("masks", [P, 4 * NT], BF16, kind="ExternalInput")
    ones_col = nc.dram_tensor("ones_col", [P, 1], F32R, kind="ExternalInput")
    ones_row = nc.dram_tensor("ones_row", [1, P], F32R, kind="ExternalInput")
    rotm = nc.dram_tensor("rotm", [P, P], BF16, kind="ExternalInput")
    out = nc.dram_tensor("out", [TS, D], F32, kind="ExternalOutput")

    RG = [list(range(NCORES))]
    HD = DR // 2

    with tile.TileContext(nc) as tc:
        with (
            tc.tile_pool(name="const", bufs=1) as cpool,
            tc.tile_pool(name="dram", bufs=1, space="DRAM") as dram,
        ):
            onc = cpool.tile([P, 1], F32R, name="onc")
            onr = cpool.tile([1, P], F32R, name="onr")
            nc.sync.dma_start(onc[:], ones_col[:])
            nc.sync.dma_start(onr[:], ones_row[:])
            mask_sb = cpool.tile([P, 4 * NT], BF16, name="mask_sb")
            nc.sync.dma_start(mask_sb[:], masks[:])
            eps_sb = cpool.tile([1, 1], F32, name="eps_sb")
            nc.vector.memset(eps_sb[:], EPS)
            rotm_sb = cpool.tile([P, P], BF16, name="rotm_sb")
            nc.sync.dma_start(rotm_sb[:], rotm[:])

            cc_kv_in = dram.tile([CKV, TS], BF16, name="cc_kv_in")
            cc_kv_out = dram.tile([NCORES * CKV, TS], BF16,
                                  addr_space="Shared", name="cc_kv_out")
            cc_q_in = dram.tile([QR, TS], BF16, name="cc_q_in")
            cc_q_out = dram.tile([NCORES * QR, TS], BF16,
                                 addr_space="Shared", name="cc_q_out")
            cc_ao_in = [dram.tile([NCORES * HL * DV, HB], BF16,
                                  name=f"cc_ao_in{b}") for b in range(B)]
            cc_ao_out = [dram.tile([NCORES * HL * DV, HB], BF16,
                                   name=f"cc_ao_out{b}") for b in range(B)]

            # =================== stage 1 =================================
            with (
                tc.tile_pool(name="s1sb", bufs=1) as s1,
                tc.tile_pool(name="s1ps", bufs=1, space="PSUM") as ps1,
            ):
                xT_sb = s1.tile([P, KD * TS], BF16, name="xT_sb")
                wkva_sb = s1.tile([P, KD * CKV], BF16, name="wkva_sb")
                wqa_sb = s1.tile([P, KD * QR], BF16, name="wqa_sb")
                for k in range(KD):
                    nc.sync.dma_start(xT_sb[:, k * TS:(k + 1) * TS],
                                      xT[k * P:(k + 1) * P, :])
                    nc.sync.dma_start(wkva_sb[:, k * CKV:(k + 1) * CKV],
                                      wkva[k * P:(k + 1) * P, :])
                    nc.sync.dma_start(wqa_sb[:, k * QR:(k + 1) * QR],
                                      wqa[k * P:(k + 1) * P, :])
                cos1_sb = s1.tile([DR, TS], F32, name="cos1_sb")
                sin1_sb = s1.tile([DR, TS], F32, name="sin1_sb")
                nc.sync.dma_start(cos1_sb[:], cos_s1[:])
                nc.sync.dma_start(sin1_sb[:], sin_s1[:])

                # ---- kv side first (its AllGather overlaps the q side)
                c_ps = []
                for m in range(MC):
                    ps = ps1.tile([P, TS], F32, tag="s1mm", bufs=4,
                                  name=f"ckv_ps{m}")
                    for k in range(KD):
                        nc.tensor.matmul(
                            ps[:],
                            wkva_sb[:, k * CKV + m * P:k * CKV + (m + 1) * P],
                            xT_sb[:, k * TS:(k + 1) * TS],
                            start=(k == 0), stop=(k == KD - 1))
                    c_ps.append(ps)
                kr_ps = ps1.tile([DR, TS], F32, tag="s1kr", bufs=2,
                                 name="kr_ps")
                for k in range(KD):
                    nc.tensor.matmul(
                        kr_ps[:],
                        wkva_sb[:, k * CKV + KVR:k * CKV + KVR + DR],
                        xT_sb[:, k * TS:(k + 1) * TS],
                        start=(k == 0), stop=(k == KD - 1))

                # rotate-half via PE: krot = rotm.T @ kr
                kraw = s1.tile([DR, TS], BF16, name="kraw")
                nc.scalar.copy(kraw[:], kr_ps[:])
                krot_ps = ps1.tile([DR, TS], F32, tag="s1kr", bufs=2,
                                   name="krot_ps")
                nc.tensor.matmul(krot_ps[:], rotm_sb[0:DR, 0:DR], kraw[:],
                                 start=True, stop=True)
                t1 = s1.tile([DR, TS], F32, name="t1")
                t2 = s1.tile([DR, TS], F32, name="t2")
                nc.vector.tensor_mul(t1[:], kr_ps[:], cos1_sb[:])
                nc.vector.tensor_mul(t2[:], krot_ps[:], sin1_sb[:])
                kro_sb = s1.tile([DR, TS], BF16, name="kro_sb")
                nc.vector.tensor_add(kro_sb[:], t1[:], t2[:])
                nc.sync.dma_start(cc_kv_in[KVR:CKV, :], kro_sb[:])

                ss_ps = ps1.tile([1, TS], F32, tag="s1row", bufs=1,
                                 name="ss_kv")
                for m in range(MC):
                    sq = s1.tile([P, TS], F32R, tag="sq", bufs=2,
                                 name=f"sqkv{m}")
                    nc.scalar.square(sq[:], c_ps[m][:])
                    nc.tensor.matmul(ss_ps[:], onc[:], sq[:],
                                     start=(m == 0), stop=(m == MC - 1))
                srow = s1.tile([1, TS], F32, tag="srow", bufs=2,
                               name="srow_kv")
                nc.scalar.activation(srow[:], ss_ps[:], AF.Sqrt,
                                     bias=eps_sb[:], scale=1.0 / KVR)
                rrow = s1.tile([1, TS], F32R, tag="rrow", bufs=2,
                               name="rrow_kv")
                with nc.allow_low_precision(reason="f32r feeds f32r matmul"):
                    nc.vector.reciprocal(rrow[:], srow[:])
                bc_ps = ps1.tile([P, TS], F32, tag="s1bc", bufs=1,
                                 name="bc_kv")
                nc.tensor.matmul(bc_ps[:], onr[:], rrow[:],
                                 start=True, stop=True)
                bc_sb = s1.tile([P, TS], F32, tag="bcs", bufs=2,
                                name="bcs_kv")
                nc.scalar.copy(bc_sb[:], bc_ps[:])
                for m in range(MC):
                    cn = s1.tile([P, TS], BF16, tag="cn", bufs=2,
                                 name=f"cn{m}")
                    nc.vector.tensor_mul(cn[:], c_ps[m][:], bc_sb[:])
                    nc.sync.dma_start(cc_kv_in[m * P:(m + 1) * P, :], cn[:])

                nc.gpsimd.collective_compute(
                    "AllGather", mybir.AluOpType.bypass, replica_groups=RG,
                    ins=[cc_kv_in.opt()], outs=[cc_kv_out.opt()])

                # ---- q side
                ssq_ps = ps1.tile([1, TS], F32, tag="s1row", bufs=1,
                                  name="ss_q")
                qa_raw = []
                for m in range(MQ):
                    ps = ps1.tile([P, TS], F32, tag="s1mm", bufs=4,
                                  name=f"qa_ps{m}")
                    for k in range(KD):
                        nc.tensor.matmul(
                            ps[:],
                            wqa_sb[:, k * QR + m * P:k * QR + (m + 1) * P],
                            xT_sb[:, k * TS:(k + 1) * TS],
                            start=(k == 0), stop=(k == KD - 1))
                    raw = s1.tile([P, TS], F32, tag=f"qraw{m}", bufs=1,
                                  name=f"qraw{m}")
                    nc.vector.tensor_copy(raw[:], ps[:])
                    qa_raw.append(raw)
                    sq = s1.tile([P, TS], F32R, tag="sq", bufs=2,
                                 name=f"sqq{m}")
                    nc.scalar.square(sq[:], ps[:])
                    nc.tensor.matmul(ssq_ps[:], onc[:], sq[:],
                                     start=(m == 0), stop=(m == MQ - 1))
                srow_q = s1.tile([1, TS], F32, tag="srow", bufs=2,
                                 name="srow_q")
                nc.scalar.activation(srow_q[:], ssq_ps[:], AF.Sqrt,
                                     bias=eps_sb[:], scale=1.0 / QR)
                rrow_q = s1.tile([1, TS], F32R, tag="rrow", bufs=2,
                                 name="rrow_q")
                with nc.allow_low_precision(reason="f32r feeds f32r matmul"):
                    nc.vector.reciprocal(rrow_q[:], srow_q[:])
                bcq_ps = ps1.tile([P, TS], F32, tag="s1bc", bufs=1,
                                  name="bc_q")
                nc.tensor.matmul(bcq_ps[:], onr[:], rrow_q[:],
                                 start=True, stop=True)
                bcq_sb = s1.tile([P, TS], F32, tag="bcs", bufs=2,
                                 name="bcs_q")
                nc.scalar.copy(bcq_sb[:], bcq_ps[:])
                for m in range(MQ):
                    qn = s1.tile([P, TS], BF16, tag="qn", bufs=2,
                                 name=f"qn{m}")
                    nc.vector.tensor_mul(qn[:], qa_raw[m][:], bcq_sb[:])
                    nc.sync.dma_start(cc_q_in[m * P:(m + 1) * P, :], qn[:])

                nc.gpsimd.collective_compute(
                    "AllGather", mybir.AluOpType.bypass, replica_groups=RG,
                    ins=[cc_q_in.opt()], outs=[cc_q_out.opt()])

            # =================== stage 2 =================================
            ckv_g = cc_kv_out.rearrange("(r p) t -> r p t", r=NCORES)
            q_g = cc_q_out.rearrange("(r p) t -> r p t", r=NCORES)

            with (
                tc.tile_pool(name="s2sb", bufs=1) as s2,
                tc.tile_pool(name="attnsb", bufs=1) as sA,
                tc.tile_pool(name="s2ps", bufs=2, space="PSUM") as ps2,
                tc.tile_pool(name="attnps", bufs=1, space="PSUM") as psA,
            ):
                kn_sb = s2.tile([P, HL * T], BF16, name="kn_sb")
                qn_sb = s2.tile([P, HL * T], BF16, name="qn_sb")
                qr_sb = s2.tile([P, T], BF16, name="qr_sb")
                # shared rotated rope key, duplicated in both partition
                # halves so lhsT/rhs base partitions match per head
                kro2_sb = s2.tile([P, T], BF16, name="kro2_sb")
                v_tiles = [s2.tile([P, HL * DV], BF16, tag=f"v{tt}", bufs=1,
                                   name=f"v{tt}") for tt in range(T // P)]

                with tc.tile_pool(name="projsb", bufs=1) as pj:
                    wqb_sb = pj.tile([P, MQ * HL * (DN + DR)], BF16,
                                     name="wqb_sb")
                    wkvbk_sb = pj.tile([P, MC * HL * DN], BF16,
                                       name="wkvbk_sb")
                    wkvbv_sb = pj.tile([P, MC * HL * DV], BF16,
                                       name="wkvbv_sb")
                    WQBC = HL * (DN + DR)
                    for k in range(MQ):
                        nc.sync.dma_start(
                            wqb_sb[:, k * WQBC:(k + 1) * WQBC],
                            wqb[k * P:(k + 1) * P, :])
                    for k in range(MC):
                        nc.sync.dma_start(
                            wkvbk_sb[:, k * HL * DN:(k + 1) * HL * DN],
                            wkvbk[k * P:(k + 1) * P, :])
                        nc.sync.dma_start(
                            wkvbv_sb[:, k * HL * DV:(k + 1) * HL * DV],
                            wkvbv[k * P:(k + 1) * P, :])
                    cos2_sb = pj.tile([P, T], BF16, name="cos2_sb")
                    sin2_sb = pj.tile([P, T], BF16, name="sin2_sb")
                    nc.sync.dma_start(cos2_sb[:], cos2[:])
                    nc.sync.dma_start(sin2_sb[:], sin2[:])

                    # assemble gathered c_kv / rope key into natural order
                    c_sb = pj.tile([P, MC * T], BF16, name="c_sb")
                    c_v = c_sb.rearrange("p (k b t) -> p k b t", k=MC, b=B)
                    kro_v0 = kro2_sb[0:DR, :].rearrange("p (b t) -> p b t",
                                                        b=B)
                    kro_v1 = kro2_sb[DR:P, :].rearrange("p (b t) -> p b t",
                                                        b=B)
                    for r in range(NCORES):
                        src = ckv_g[r].rearrange("p (b t) -> p b t", b=B)
                        for k in range(MC):
                            nc.sync.dma_start(
                                c_v[:, k, :, HB * r:HB * (r + 1)],
                                src[k * P:(k + 1) * P, :, :])
                        nc.sync.dma_start(
                            kro_v0[:, :, HB * r:HB * (r + 1)],
                            src[KVR:CKV, :, :])
                        nc.sync.dma_start(
                            kro_v1[:, :, HB * r:HB * (r + 1)],
                            src[KVR:CKV, :, :])

                    # kT projection
                    for h in range(HL):
                        for n in range(NTT):
                            ps = ps2.tile([P, NT], F32, tag="proj",
                                          name="kn_ps")
                            for k in range(MC):
                                nc.tensor.matmul(
                                    ps[:],
                                    wkvbk_sb[:, k * HL * DN + h * DN:
                                             k * HL * DN + (h + 1) * DN],
                                    c_sb[:, k * T + n * NT:
                                         k * T + (n + 1) * NT],
                                    start=(k == 0), stop=(k == MC - 1))
                            nc.scalar.copy(
                                kn_sb[:, h * T + n * NT:h * T + (n + 1) * NT],
                                ps[:])

                    # v projection (natural layout)
                    for tt in range(T // P):
                        ps = ps2.tile([P, HL * DV], F32, tag="proj",
                                      name="v_ps")
                        for k in range(MC):
                            nc.tensor.matmul(
                                ps[:],
                                c_sb[:, k * T + tt * P:k * T + (tt + 1) * P],
                                wkvbv_sb[:, k * HL * DV:(k + 1) * HL * DV],
                                start=(k == 0), stop=(k == MC - 1))
                        nc.scalar.copy(v_tiles[tt][:], ps[:])

                    # qT projection + rope
                    for n in range(NTT):
                        jq, bq = n % 4, n // 4
                        rhs = []
                        for k in range(MQ):
                            qt = pj.tile([P, NT], BF16, tag="qrhs", bufs=16,
                                         name=f"qrhs{k}")
                            dst = qt.rearrange("p (r t) -> p r t", r=2)
                            src = q_g[2 * jq:2 * jq + 2, k * P:(k + 1) * P,
                                      bq * HB:(bq + 1) * HB]
                            nc.sync.dma_start(dst[:],
                                              src.rearrange("r p t -> p r t"))
                            rhs.append(qt)
                        for m in range(3):
                            ps = ps2.tile([P, NT], F32, tag="proj",
                                          name="q_ps")
                            for k in range(MQ):
                                nc.tensor.matmul(
                                    ps[:],
                                    wqb_sb[:, k * WQBC + m * P:
                                           k * WQBC + (m + 1) * P],
                                    rhs[k][:],
                                    start=(k == 0), stop=(k == MQ - 1))
                            if m < HL:
                                nc.scalar.copy(
                                    qn_sb[:, m * T + n * NT:
                                          m * T + (n + 1) * NT], ps[:])
                            else:
                                ct = cos2_sb[:, n * NT:(n + 1) * NT]
                                st = sin2_sb[:, n * NT:(n + 1) * NT]
                                qraw = pj.tile([P, NT], BF16, tag="qraw",
                                               bufs=2, name="qraw")
                                nc.scalar.copy(qraw[:], ps[:])
                                rps = ps2.tile([P, NT], F32, tag="proj",
                                               name="rps")
                                nc.tensor.matmul(rps[:], rotm_sb[:], qraw[:],
                                                 start=True, stop=True)
                                u1 = pj.tile([P, NT], F32, tag="u1", bufs=2,
                                             name="u1")
                                u2 = pj.tile([P, NT], F32, tag="u2", bufs=2,
                                             name="u2")
                                nc.vector.tensor_mul(u1[:], ps[:], ct)
                                nc.vector.tensor_mul(u2[:], rps[:], st)
                                nc.vector.tensor_add(
                                    qr_sb[:, n * NT:(n + 1) * NT],
                                    u1[:], u2[:])

                # ---- attention ----
                ao_sb = [sA.tile([DV, T], BF16, name=f"ao_sb{h}")
                         for h in range(HL)]
                for b in range(B):
                    for h in range(HL):
                        for qti in range(SB):
                            qs = b * S + qti * NT
                            aop = psA.tile([DV, NT], F32, tag="ao", bufs=2,
                                           name="aop")
                            dn = sA.tile([P, NT], F32R, tag="dn", bufs=2,
                                         name="dn")
                            nk = 4 * qti + 4
                            for kti in range(nk):
                                ks = b * S + kti * P
                                scp = psA.tile([P, NT], F32, tag="sc",
                                               bufs=2, name="scp")
                                nc.tensor.matmul(
                                    scp[:],
                                    kn_sb[:, h * T + ks:h * T + ks + P],
                                    qn_sb[:, h * T + qs:h * T + qs + NT],
                                    start=True, stop=False)
                                nc.tensor.matmul(
                                    scp[:],
                                    kro2_sb[h * DR:(h + 1) * DR, ks:ks + P],
                                    qr_sb[h * DR:(h + 1) * DR, qs:qs + NT],
                                    start=False, stop=True)
                                et = sA.tile([P, NT], BF16, tag="et", bufs=3,
                                             name="et")
                                nc.scalar.activation(et[:], scp[:], AF.Exp)
                                if kti >= 4 * qti:
                                    jm = kti % 4
                                    nc.vector.tensor_mul(
                                        et[:], et[:],
                                        mask_sb[:, jm * NT:(jm + 1) * NT])
                                if kti == 0:
                                    nc.vector.tensor_copy(dn[:], et[:])
                                else:
                                    nc.vector.tensor_add(dn[:], dn[:], et[:])
                                tt = (b * S) // P + kti
                                nc.tensor.matmul(
                                    aop[:],
                                    v_tiles[tt][:, h * DV:(h + 1) * DV],
                                    et[:],
                                    start=(kti == 0), stop=(kti == nk - 1))
                            dps = psA.tile([1, NT], F32, tag="drow", bufs=1,
                                           name="dps")
                            nc.tensor.matmul(dps[:], onc[:], dn[:],
                                             start=True, stop=True)
                            rec = sA.tile([1, NT], F32R, tag="rec", bufs=2,
                                          name="rec")
                            with nc.allow_low_precision(
                                    reason="f32r feeds f32r matmul"):
                                nc.vector.reciprocal(rec[:], dps[:])
                            bcp = psA.tile([P, NT], F32, tag="bc", bufs=1,
                                           name="bcp")
                            nc.tensor.matmul(bcp[:], onr[:], rec[:],
                                             start=True, stop=True)
                            bcs = sA.tile([P, NT], F32, tag="bcs2", bufs=2,
                                          name="bcs")
                            nc.scalar.copy(bcs[:], bcp[:])
                            nc.vector.tensor_mul(
                                ao_sb[h][:, qs:qs + NT], aop[:], bcs[:])
                    # AllToAll this batch: shard j = (my heads, core j toks)
                    for j in range(NCORES):
                        for h in range(HL):
                            nc.sync.dma_start(
                                cc_ao_in[b][(j * HL + h) * DV:
                                            (j * HL + h + 1) * DV, :],
                                ao_sb[h][:, b * S + HB * j:
                                         b * S + HB * (j + 1)])
                    nc.gpsimd.collective_compute(
                        "AllToAll", mybir.AluOpType.bypass, replica_groups=RG,
                        ins=[cc_ao_in[b].opt()], outs=[cc_ao_out[b].opt()])

                # ---- Wo: out[my 512 toks, D], full contraction ----
                with tc.tile_pool(name="wosb", bufs=1) as wp:
                    wo_sb = wp.tile([P, KO * D], BF16, name="wo_sb")
                    for k in range(KO):
                        nc.sync.dma_start(wo_sb[:, k * D:(k + 1) * D],
                                          wo[k * P:(k + 1) * P, :])
                    for b in range(B):
                        for mb in range(HB // P):        # 2 m-tiles per batch
                            aog = []
                            for k in range(KO):
                                ag = wp.tile([P, P], BF16, tag="aog",
                                             bufs=KO + 2, name=f"aog{k}")
                                nc.sync.dma_start(
                                    ag[:],
                                    cc_ao_out[b][k * P:(k + 1) * P,
                                                 mb * P:(mb + 1) * P])
                                aog.append(ag)
                            for n in range(D // NT):
                                ps = ps2.tile([P, NT], F32, tag="proj",
                                              name="wo_ps")
                                for k in range(KO):
                                    nc.tensor.matmul(
                                        ps[:], aog[k][:],
                                        wo_sb[:, k * D + n * NT:
                                              k * D + (n + 1) * NT],
                                        start=(k == 0), stop=(k == KO - 1))
                                ob = wp.tile([P, NT], F32, tag="ob", bufs=3,
                                             name="ob")
                                nc.scalar.copy(ob[:], ps[:])
                                nc.sync.dma_start(
                                    out[(b * 2 + mb) * P:
                                        (b * 2 + mb + 1) * P,
                                        n * NT:(n + 1) * NT], ob[:])

    if do_compile:
        nc.compile()
    return nc


# ---------------------------------------------------------------- entry

def _get_nc():
    if "nc" not in _CACHE:
        _CACHE["nc"] = build_nc()
    return _CACHE["nc"]


def kernel(**inputs):
    nc = _get_nc()
    in_maps = host_prep(inputs)
    res = run_bass_kernel_spmd(nc, in_maps, core_ids=list(range(NCORES)))
    outg = np.zeros((T, D), np.float32)
    for c in range(NCORES):
        outg[_shard_tokens(c)] = res.results[c]["out"]
    return outg.reshape(B, S, D)

